# revision 5
# baseline (speedup 1.0000x reference)
"""Trainium2 Bass kernel for nn_MinEuclideanDistBlock.

Problem: x [32, 8, 2048] f32, shapelets [8, 256, 64] f32.
  W = 2048 - 64 + 1 = 1985 sliding windows.
  sq[b,c,w,k] = ||x[b,c,w:w+64] - shapelets[c,k]||^2
  out[b,0,k]  = min_w sum_c sqrt(sq[b,c,w,k])

Strategy (data-parallel over batch B across 8 cores, 4 batches/core):
  - Per (b, c): one fused PE matmul computes
        psum[k, w] = sum_s sh[c,k,s] * x[b,c,w+s]  -  x2[b,c,w]/2
    using a 66-row contraction: rows 0..63 are a Hankel view of x
    (built by a single overlapping strided DMA, bf16), rows 64..65 carry
    -x2/2 split into bf16 hi+lo for precision. lhsT rows 64..65 are ones.
  - ACT computes d = sqrt(-2*psum + s2[k]) with s2 as the per-partition
    activation bias (fp32, exact).  This is the compute bottleneck.
  - DVE accumulates d over the 8 channels in fp32; the last add is fused
    with the min-over-w reduction via tensor_tensor_reduce.
  - x2 (window energy) via 6 log-step shifted adds on DVE.
"""

import sys

for _p in ("/opt/trn_rl_repo",):
    if _p not in sys.path:
        sys.path.insert(0, _p)

import numpy as np

import concourse.bass as bass
import concourse.bacc as bacc
import concourse.mybir as mybir
import concourse.tile as tile
from concourse.ap import AP
from concourse.bass_utils import run_bass_kernel_spmd

# Problem constants (hardcoded per the harness contract).
B, C, L = 32, 8, 2048
S, K = 64, 256
W = L - S + 1  # 1985
NCORES = 8
BLOC = B // NCORES  # 4 batches per core
KH = 2  # two halves of K=256, 128 partitions each
CHUNK = 512
CHUNKS = [(j * CHUNK, min(CHUNK, W - j * CHUNK)) for j in range((W + CHUNK - 1) // CHUNK)]

FP32 = mybir.dt.float32
BF16 = mybir.dt.bfloat16


def build_program():
    nc = bacc.Bacc("TRN2", target_bir_lowering=False, debug=False,
                   enable_asserts=False, num_devices=NCORES)

    x_dram = nc.dram_tensor("x", [BLOC, C, L], FP32, kind="ExternalInput")
    sh_dram = nc.dram_tensor("sh", [C, K, S], FP32, kind="ExternalInput")
    out_dram = nc.dram_tensor("out", [BLOC, 1, K], FP32, kind="ExternalOutput")
    # bf16 staging copy of x in DRAM; source for the Hankel DMAs.
    xbf_dram = nc.dram_tensor("xbf", [BLOC, C, L], BF16, kind="Internal")

    with tile.TileContext(nc) as tc:
        with tc.tile_pool(name="const", bufs=1) as const_pool:
            # ---- persistent setup tiles ----
            # weights: [66, 2048] bf16; cols (c*2+kh)*128 + k_local; rows 0..63 =
            # sh[c, k, s] (s on partitions), rows 64..65 = 1.0
            wts = const_pool.tile([S + 2, C * K], BF16)
            # s2[c*2+kh partitioned by k_local]: [128, 16] f32, col i=(c*2+kh)
            s2 = const_pool.tile([128, C * KH], FP32)
            # x2 hi/lo rows: [32, 2*2048] bf16; [p=b*C+c, 0:W] = bf16(-x2/2),
            # [p, 2048:2048+W] = residual
            x2pack = const_pool.tile([BLOC * C, 2 * L], BF16)

            setup_ctx = tc.tile_pool(name="setup", bufs=1)
            setup_pool = setup_ctx.__enter__()
            # ---- load x, cast to bf16, stage to DRAM ----
            xs = setup_pool.tile([BLOC * C, L], FP32)
            nc.sync.dma_start(xs[:, :], x_dram[:].flatten_outer_dims())
            xbf_s = setup_pool.tile([BLOC * C, L], BF16)
            nc.vector.tensor_copy(xbf_s[:, :], xs[:, :])
            nc.sync.dma_start(xbf_dram[:].flatten_outer_dims(), xbf_s[:, :])

            # ---- x2 = sliding window energy, via log-step shifted adds ----
            xsq = setup_pool.tile([BLOC * C, L], FP32)
            nc.scalar.square(xsq[:, :], xs[:, :])
            ta = setup_pool.tile([BLOC * C, L], FP32)
            tb = setup_pool.tile([BLOC * C, L], FP32)
            cur, nxt = xsq, ta
            n = L
            for shift in (1, 2, 4, 8, 16):
                n -= shift
                nc.vector.tensor_add(nxt[:, 0:n], cur[:, 0:n], cur[:, shift:shift + n])
                cur, nxt = nxt, (tb if nxt is ta else ta)
            # last pass: n becomes W; then fold in the -0.5 scale
            assert n - 32 == W
            x2f = setup_pool.tile([BLOC * C, W], FP32)
            nc.vector.tensor_add(x2f[:, 0:W], cur[:, 0:W], cur[:, 32:32 + W])
            y = setup_pool.tile([BLOC * C, W], FP32)  # y = -x2/2
            nc.vector.tensor_scalar_mul(y[:, 0:W], x2f[:, 0:W], -0.5)
            # hi/lo split to bf16
            nc.vector.tensor_copy(x2pack[:, 0:W], y[:, 0:W])
            nc.vector.tensor_sub(x2pack[:, L:L + W], y[:, 0:W], x2pack[:, 0:W])

            # ---- shapelet weights (transposed via strided DMA gather) ----
            shg = setup_pool.tile([S, C * K], FP32)
            # shg[s, kg] = sh[kg*64 + s]; partition step 1, free step 64
            nc.sync.dma_start(shg[:, :], AP(sh_dram, 0, [[1, S], [S, C * K]]))
            nc.vector.tensor_copy(wts[0:S, :], shg[:, :])
            nc.vector.memset(wts[S:S + 2, :], 1.0)

            # ---- s2 = per-shapelet energy ----
            sh_flat = sh_dram[:].flatten_outer_dims()  # [2048, 64]
            for i in range(C * KH):
                shs = setup_pool.tile([128, S], FP32, name="shs")
                nc.sync.dma_start(shs[:, :], sh_flat[i * 128:(i + 1) * 128, :])
                shsq = setup_pool.tile([128, S], FP32, name="shsq")
                nc.scalar.square(shsq[:, :], shs[:, :])
                nc.vector.tensor_reduce(s2[:, i:i + 1], shsq[:, :],
                                        axis=mybir.AxisListType.X,
                                        op=mybir.AluOpType.add)

            setup_ctx.__exit__(None, None, None)

            # ---- main loop ----
            with (
                tc.tile_pool(name="rhs", bufs=3) as rhs_pool,
                tc.tile_pool(name="psum", bufs=2, space=bass.MemorySpace.PSUM) as psum_pool,
                tc.tile_pool(name="dtmp", bufs=3) as dtmp_pool,
                tc.tile_pool(name="acc", bufs=4) as acc_pool,
                tc.tile_pool(name="mcol", bufs=2 * KH) as mcol_pool,
            ):
                for b in range(BLOC):
                    accs = [acc_pool.tile([128, W], FP32, name=f"acc{kh}", tag=f"acc{kh}")
                            for kh in range(KH)]
                    for c in range(C):
                        bc = b * C + c
                        rhs = rhs_pool.tile([S + 2, L], BF16, name="rhs", tag="rhs")
                        # Hankel rows: rhs[s, w] = xbf[b, c, w + s]
                        nc.sync.dma_start(
                            rhs[0:S, 0:W],
                            AP(xbf_dram, bc * L, [[1, S], [1, W]]),
                        )
                        # x2 rows (hi, lo)
                        nc.sync.dma_start(
                            rhs[S:S + 2, 0:W],
                            x2pack[bc:bc + 1, :].rearrange(
                                "p (two n) -> p two n", two=2)[:, :, 0:W],
                        )
                        for kh in range(KH):
                            i = c * KH + kh
                            psum = psum_pool.tile([128, 2048], FP32, name="psum",
                                                  tag="psum")
                            for (w0, wn) in CHUNKS:
                                nc.tensor.matmul(
                                    psum[:, w0:w0 + wn],
                                    wts[:, i * 128:(i + 1) * 128],
                                    rhs[:, w0:w0 + wn],
                                    start=True, stop=True,
                                )
                            # d = sqrt(-2*psum + s2)
                            if c == 0:
                                nc.scalar.activation(
                                    accs[kh][:, 0:W], psum[:, 0:W],
                                    mybir.ActivationFunctionType.Sqrt,
                                    bias=s2[:, i:i + 1], scale=-2.0)
                            else:
                                d = dtmp_pool.tile([128, W], FP32, name="d", tag="d")
                                nc.scalar.activation(
                                    d[:, 0:W], psum[:, 0:W],
                                    mybir.ActivationFunctionType.Sqrt,
                                    bias=s2[:, i:i + 1], scale=-2.0)
                                nc.vector.tensor_add(
                                    accs[kh][:, 0:W], accs[kh][:, 0:W], d[:, 0:W])
                                if c == C - 1:
                                    mcol = mcol_pool.tile([128, 1], FP32,
                                                          name="mcol", tag="mcol")
                                    nc.vector.tensor_reduce(
                                        mcol[:, 0:1], accs[kh][:, 0:W],
                                        axis=mybir.AxisListType.X,
                                        op=mybir.AluOpType.min,
                                    )
                                    nc.sync.dma_start(
                                        out_dram[b, 0, kh * 128:(kh + 1) * 128],
                                        mcol[:, 0:1],
                                    )

    nc.compile()
    return nc


_PROGRAM_CACHE = {}


def kernel(x: np.ndarray, shapelets: np.ndarray) -> np.ndarray:
    x = np.ascontiguousarray(np.asarray(x, dtype=np.float32))
    shapelets = np.ascontiguousarray(np.asarray(shapelets, dtype=np.float32))
    assert x.shape == (B, C, L) and shapelets.shape == (C, K, S)

    if "nc" not in _PROGRAM_CACHE:
        _PROGRAM_CACHE["nc"] = build_program()
    nc = _PROGRAM_CACHE["nc"]

    in_maps = [
        {"x": x[i * BLOC:(i + 1) * BLOC], "sh": shapelets}
        for i in range(NCORES)
    ]
    results = run_bass_kernel_spmd(nc, in_maps, core_ids=list(range(NCORES))).results
    out = np.concatenate([results[i]["out"] for i in range(NCORES)], axis=0)
    return out.astype(np.float32)


if __name__ == "__main__":
    rng = np.random.default_rng(0)
    xt = rng.standard_normal((B, C, L), dtype=np.float32)
    st = rng.standard_normal((C, K, S), dtype=np.float32)
    o = kernel(xt, st)
    print("kernel output shape:", o.shape, o.dtype)


# revision 16
# speedup vs baseline: 615.7960x; 615.7960x over previous
"""Trainium2 Bass kernel for nn_MinEuclideanDistBlock.

Problem: x [32, 8, 2048] f32, shapelets [8, 256, 64] f32.
  W = 2048 - 64 + 1 = 1985 sliding windows.
  sq[b,c,w,k] = ||x[b,c,w:w+64] - shapelets[c,k]||^2
  out[b,0,k]  = min_w sum_c sqrt(sq[b,c,w,k])

Strategy (data-parallel over batch B across 8 cores, 4 batches/core):
  - Per (b, c): fused PE matmuls (bf16 in, fp32 PSUM) compute
        psum[k, w] = sum_s sh[c,k,s] * x[b,c,w+s]  -  x2[b,c,w]/2
    using a 66-row contraction: rows 0..63 are a Hankel view of x
    (built by a single overlapping strided DMA from a bf16 staging copy),
    rows 64..65 carry -x2/2 split into bf16 hi+lo for precision.
    lhsT rows 64..65 are ones.
  - ACT computes d = sqrt(-2*psum + s2[k]) with s2 as the per-partition
    activation bias (fp32, exact).  This is the compute bottleneck
    (~115 us busy per core).
  - DVE accumulates d over the 8 channels in fp32, then reduces min over
    the 1985 windows.
  - x2 (window energy) via log-step shifted adds on DVE.
  - Shapelet weights are transposed on-chip via PE (identity matmul) —
    a strided gather DMA here costs ~1 ms in DMA descriptors.
  - Measured steady-state: ~144 us per batch-group iteration per core;
    max relative error vs the fp32 reference: 2.4e-4.

Note: tensor_tensor_reduce faults TRN2 hardware in this environment
(wedges the device); use separate tensor_tensor + tensor_reduce.
"""

import sys

for _p in ("/opt/trn_rl_repo",):
    if _p not in sys.path:
        sys.path.insert(0, _p)

import numpy as np

import concourse.bass as bass
import concourse.bacc as bacc
import concourse.mybir as mybir
import concourse.tile as tile
from concourse.ap import AP
from concourse.bass_utils import run_bass_kernel_spmd

# Problem constants (hardcoded per the harness contract).
B, C, L = 32, 8, 2048
S, K = 64, 256
W = L - S + 1  # 1985
NCORES = 8
BLOC = B // NCORES  # 4 batches per core
KH = 2  # two halves of K=256, 128 partitions each
CHUNK = 512
CHUNKS = [(j * CHUNK, min(CHUNK, W - j * CHUNK)) for j in range((W + CHUNK - 1) // CHUNK)]

FP32 = mybir.dt.float32
BF16 = mybir.dt.bfloat16

# Channel-accumulation strategy:
#   "serial_fp32": d in fp32, 7 serial DVE adds (1x mode)  — max precision
#   "tree_bf16":   d in bf16, pairwise tree adds (2x mode) — ~2x faster DVE
ADDS = "serial_fp32"
# dtype for the d tiles / pair sums in the tree variants
TREE_DT = {"tree_bf16": BF16, "tree_fp32": FP32}.get(ADDS, FP32)
TREE_2B = TREE_DT in (BF16,)
D_BUFS = 8 if TREE_2B else 7
PS_BUFS = 10 if TREE_2B else 8


def build_program(reps: int = 1):
    nc = bacc.Bacc("TRN2", target_bir_lowering=False, debug=False,
                   enable_asserts=False, num_devices=NCORES)

    x_dram = nc.dram_tensor("x", [BLOC, C, L], FP32, kind="ExternalInput")
    sh_dram = nc.dram_tensor("sh", [C, K, S], FP32, kind="ExternalInput")
    out_dram = nc.dram_tensor("out", [BLOC, 1, K], FP32, kind="ExternalOutput")
    # bf16 staging copy of x in DRAM; source for the Hankel DMAs.
    xbf_dram = nc.dram_tensor("xbf", [BLOC, C, L], BF16, kind="Internal")

    with tile.TileContext(nc) as tc:
        with tc.tile_pool(name="const", bufs=1) as const_pool:
            # ---- persistent setup tiles ----
            # weights: [66, 2048] bf16; cols (c*2+kh)*128 + k_local; rows 0..63 =
            # sh[c, k, s] (s on partitions), rows 64..65 = 1.0
            wts = const_pool.tile([S + 2, C * K], BF16)
            # s2[c*2+kh partitioned by k_local]: [128, 16] f32, col i=(c*2+kh)
            s2 = const_pool.tile([128, C * KH], FP32)
            # x2 hi/lo rows: [32, 2*2048] bf16; [p=b*C+c, 0:W] = bf16(-x2/2),
            # [p, 2048:2048+W] = residual
            x2pack = const_pool.tile([BLOC * C, 2 * L], BF16)

            setup_ctx = tc.tile_pool(name="setup", bufs=1)
            setup_pool = setup_ctx.__enter__()
            # ---- load x, cast to bf16, stage to DRAM ----
            xs = setup_pool.tile([BLOC * C, L], FP32)
            nc.sync.dma_start(xs[:, :], x_dram[:].flatten_outer_dims())
            xbf_s = setup_pool.tile([BLOC * C, L], BF16)
            nc.vector.tensor_copy(xbf_s[:, :], xs[:, :])
            nc.sync.dma_start(xbf_dram[:].flatten_outer_dims(), xbf_s[:, :])

            # ---- x2 = sliding window energy, via log-step shifted adds ----
            xsq = setup_pool.tile([BLOC * C, L], FP32)
            nc.scalar.square(xsq[:, :], xs[:, :])
            ta = setup_pool.tile([BLOC * C, L], FP32)
            tb = setup_pool.tile([BLOC * C, L], FP32)
            cur, nxt = xsq, ta
            n = L
            for shift in (1, 2, 4, 8, 16):
                n -= shift
                nc.vector.tensor_add(nxt[:, 0:n], cur[:, 0:n], cur[:, shift:shift + n])
                cur, nxt = nxt, (tb if nxt is ta else ta)
            # last pass: n becomes W; then fold in the -0.5 scale
            assert n - 32 == W
            x2f = setup_pool.tile([BLOC * C, W], FP32)
            nc.vector.tensor_add(x2f[:, 0:W], cur[:, 0:W], cur[:, 32:32 + W])
            y = setup_pool.tile([BLOC * C, W], FP32)  # y = -x2/2
            nc.vector.tensor_scalar_mul(y[:, 0:W], x2f[:, 0:W], -0.5)
            # hi/lo split to bf16
            nc.vector.tensor_copy(x2pack[:, 0:W], y[:, 0:W])
            nc.vector.tensor_sub(x2pack[:, L:L + W], y[:, 0:W], x2pack[:, 0:W])

            # ---- shapelet weights (sequential load + on-chip PE transpose;
            #      a strided DMA gather here costs ~1 ms in descriptors) ----
            from concourse import masks
            ident = setup_pool.tile([128, 128], BF16)
            masks.make_identity(nc, ident[:, :])
            nc.vector.memset(wts[S:S + 2, :], 1.0)
            tp_ctx = tc.tile_pool(name="tpsum", bufs=2,
                                  space=bass.MemorySpace.PSUM)
            tp_pool = tp_ctx.__enter__()

            # ---- s2 = per-shapelet energy + transposed bf16 weights ----
            sh_flat = sh_dram[:].flatten_outer_dims()  # [2048, 64]
            for i in range(C * KH):
                shs = setup_pool.tile([128, S], FP32, name="shs")
                nc.sync.dma_start(shs[:, :], sh_flat[i * 128:(i + 1) * 128, :])
                shsq = setup_pool.tile([128, S], FP32, name="shsq")
                nc.scalar.square(shsq[:, :], shs[:, :])
                nc.vector.tensor_reduce(s2[:, i:i + 1], shsq[:, :],
                                        axis=mybir.AxisListType.X,
                                        op=mybir.AluOpType.add)
                shb = setup_pool.tile([128, S], BF16, name="shb")
                nc.vector.tensor_copy(shb[:, :], shs[:, :])
                shT = tp_pool.tile([S, 128], BF16, name="shT")
                nc.tensor.transpose(shT[:, :], shb[:, :], ident[:, :])
                nc.vector.tensor_copy(wts[0:S, i * 128:(i + 1) * 128], shT[:, :])
            tp_ctx.__exit__(None, None, None)

            setup_ctx.__exit__(None, None, None)

            # ---- main loop ----
            with (
                tc.tile_pool(name="rhs", bufs=3) as rhs_pool,
                tc.tile_pool(name="psum", bufs=2, space=bass.MemorySpace.PSUM) as psum_pool,
                tc.tile_pool(name="dtmp", bufs=8) as dtmp_pool,
                tc.tile_pool(name="acc", bufs=4) as acc_pool,
                tc.tile_pool(name="mcol", bufs=2 * KH) as mcol_pool,
            ):
                for rep_b in range(reps * BLOC):
                    b = rep_b % BLOC
                    if ADDS == "serial_fp32":
                        accs = [acc_pool.tile([128, W], FP32, name=f"acc{kh}",
                                              tag=f"acc{kh}")
                                for kh in range(KH)]
                    else:
                        dts = {}    # (kh, c) -> bf16 d tile
                        sums = {}   # (kh, lvl_key) -> partial sums
                    for c in range(C):
                        bc = b * C + c
                        rhs = rhs_pool.tile([S + 2, L], BF16, name="rhs", tag="rhs")
                        # Hankel rows: rhs[s, w] = xbf[b, c, w + s]
                        nc.sync.dma_start(
                            rhs[0:S, 0:W],
                            AP(xbf_dram, bc * L, [[1, S], [1, W]]),
                        )
                        # x2 rows (hi, lo)
                        nc.sync.dma_start(
                            rhs[S:S + 2, 0:W],
                            x2pack[bc:bc + 1, :].rearrange(
                                "p (two n) -> p two n", two=2)[:, :, 0:W],
                        )
                        for kh in range(KH):
                            i = c * KH + kh
                            psum = psum_pool.tile([128, 2048], FP32, name="psum",
                                                  tag="psum")
                            for (w0, wn) in CHUNKS:
                                nc.tensor.matmul(
                                    psum[:, w0:w0 + wn],
                                    wts[:, i * 128:(i + 1) * 128],
                                    rhs[:, w0:w0 + wn],
                                    start=True, stop=True,
                                )
                            # d = sqrt(-2*psum + s2)
                            if ADDS == "serial_fp32":
                                if c == 0:
                                    nc.scalar.activation(
                                        accs[kh][:, 0:W], psum[:, 0:W],
                                        mybir.ActivationFunctionType.Sqrt,
                                        bias=s2[:, i:i + 1], scale=-2.0)
                                else:
                                    d = dtmp_pool.tile([128, W], FP32, name="d",
                                                       tag="d")
                                    nc.scalar.activation(
                                        d[:, 0:W], psum[:, 0:W],
                                        mybir.ActivationFunctionType.Sqrt,
                                        bias=s2[:, i:i + 1], scale=-2.0)
                                    nc.vector.tensor_add(
                                        accs[kh][:, 0:W], accs[kh][:, 0:W],
                                        d[:, 0:W])
                                    if c == C - 1:
                                        mcol = mcol_pool.tile(
                                            [128, 1], FP32, name="mcol", tag="mcol")
                                        nc.vector.tensor_reduce(
                                            mcol[:, 0:1], accs[kh][:, 0:W],
                                            axis=mybir.AxisListType.X,
                                            op=mybir.AluOpType.min,
                                        )
                                        nc.sync.dma_start(
                                            out_dram[b, 0,
                                                     kh * 128:(kh + 1) * 128],
                                            mcol[:, 0:1],
                                        )
                            else:
                                d = dtmp_pool.tile([128, W], TREE_DT, name="d", tag="d",
                                                   bufs=D_BUFS)
                                nc.scalar.activation(
                                    d[:, 0:W], psum[:, 0:W],
                                    mybir.ActivationFunctionType.Sqrt,
                                    bias=s2[:, i:i + 1], scale=-2.0)
                                dts[(kh, c)] = d
                                if c % 2 == 1:
                                    # pair-sum as soon as a pair is complete
                                    s = acc_pool.tile([128, W], TREE_DT,
                                                      name="ps", tag="ps",
                                                      bufs=PS_BUFS)
                                    nc.vector.tensor_add(
                                        s[:, 0:W], dts[(kh, c - 1)][:, 0:W],
                                        d[:, 0:W])
                                    sums[(kh, c // 2)] = s
                                if c == 3:
                                    s0123 = acc_pool.tile([128, W], TREE_DT,
                                                          name="s0123", tag="ps",
                                                          bufs=PS_BUFS)
                                    nc.vector.tensor_add(
                                        s0123[:, 0:W], sums[(kh, 0)][:, 0:W],
                                        sums[(kh, 1)][:, 0:W])
                                    sums[(kh, "0123")] = s0123
                                if c == C - 1:
                                    s4567 = acc_pool.tile([128, W], TREE_DT,
                                                          name="s4567", tag="ps",
                                                          bufs=PS_BUFS)
                                    nc.vector.tensor_add(
                                        s4567[:, 0:W], sums[(kh, 2)][:, 0:W],
                                        sums[(kh, 3)][:, 0:W])
                                    tot = acc_pool.tile([128, W], FP32,
                                                        name="tot", tag="tot",
                                                        bufs=4)
                                    nc.vector.tensor_add(
                                        tot[:, 0:W], sums[(kh, "0123")][:, 0:W],
                                        s4567[:, 0:W])
                                    mcol = mcol_pool.tile([128, 1], FP32,
                                                          name="mcol", tag="mcol")
                                    nc.vector.tensor_reduce(
                                        mcol[:, 0:1], tot[:, 0:W],
                                        axis=mybir.AxisListType.X,
                                        op=mybir.AluOpType.min,
                                    )
                                    nc.sync.dma_start(
                                        out_dram[b, 0, kh * 128:(kh + 1) * 128],
                                        mcol[:, 0:1],
                                    )

    nc.compile()
    return nc


_PROGRAM_CACHE = {}


def kernel(x: np.ndarray, shapelets: np.ndarray) -> np.ndarray:
    x = np.ascontiguousarray(np.asarray(x, dtype=np.float32))
    shapelets = np.ascontiguousarray(np.asarray(shapelets, dtype=np.float32))
    assert x.shape == (B, C, L) and shapelets.shape == (C, K, S)

    if "nc" not in _PROGRAM_CACHE:
        _PROGRAM_CACHE["nc"] = build_program()
    nc = _PROGRAM_CACHE["nc"]

    in_maps = [
        {"x": x[i * BLOC:(i + 1) * BLOC], "sh": shapelets}
        for i in range(NCORES)
    ]
    results = run_bass_kernel_spmd(nc, in_maps, core_ids=list(range(NCORES))).results
    out = np.concatenate([results[i]["out"] for i in range(NCORES)], axis=0)
    return out.astype(np.float32)


if __name__ == "__main__":
    rng = np.random.default_rng(0)
    xt = rng.standard_normal((B, C, L), dtype=np.float32)
    st = rng.standard_normal((C, K, S), dtype=np.float32)
    o = kernel(xt, st)
    print("kernel output shape:", o.shape, o.dtype)
